# revision 45
# baseline (speedup 1.0000x reference)
"""Trainium2 Bass kernel for nn_ClassifyLayer (masked cosine-similarity classify layer).

Reference math (head==1, len_prd=4096, N=8192, D=512):
  Wb     = W[:4096, 4096:]                     (0/1 ints)
  mask   = -99 * (1 - Wb)
  ne     = embeddings / ||embeddings||_row
  sim    = ne[:4096] @ ne[4096:].T             [4096, 4096]
  masked = sim + mask
  out1   = clamp01(relu(masked))
  half   = 1 / (1 - masked)
  ext    = [half / rowsum(half) (zeroed where no positive in row), supple_flag]

Sharding: row-parallel over prd across 8 cores (512 rows/core); the det block
(embeddings[4096:], transposed host-side) is replicated to every core.

On-chip per core:
  PSUM = sum_d eprdT[d,i] * (edetT[d,j]*c_j)  (4x float32r matmuls, K=128 each)
       + sum_k diag(99*n_i)[k,i] * (W-1)[k,j] (1x bf16 matmul)  == masked / r_i
  ACT:  out1 = Relu(r_i * PSUM)           (+accum -> "any positive" flag)
        t    = 1 - r_i * PSUM             (Copy, scale=-r_i, bias=1)
  DVE:  half = reciprocal_approx_fast(t); rowsum accum; ext = half * s_row
where n_i = ||prd row i||, r_i = 1/n_i, c_j = 1/||det row j||.
"""

import os
import sys

sys.path.insert(0, "/opt/trn_rl_repo")

import numpy as np
import ml_dtypes

N_TOT, D, LP = 8192, 512, 4096
ND = N_TOT - LP          # 4096 det rows
C = 8                    # cores
RPC = LP // C            # 512 prd rows per core
RT = RPC // 128          # 4 row-tiles per core
JB = ND // 512           # 8 col-blocks of 512
NEG = 99.0

_cache = {}


def _install_ntff_hook_shim():
    """This image's `antenv` lacks `axon_hooks`; provide it so
    run_bass_kernel_spmd(trace=True) can capture NTFF profiles via the
    axon PJRT .so (same ctypes dance as trn_agent_boot)."""
    import types
    import ctypes
    import contextlib

    if "antenv.axon_hooks" in sys.modules:
        return
    so_path = "/opt/axon/libaxon_pjrt.so"
    hook = None
    if os.path.exists(so_path):
        lib = ctypes.CDLL(so_path)
        if hasattr(lib, "axon_start_nrt_profile"):
            lib.axon_start_nrt_profile.argtypes = [
                ctypes.POINTER(ctypes.c_int64),
                ctypes.c_size_t,
            ]
            lib.axon_start_nrt_profile.restype = ctypes.c_int64
            lib.axon_stop_nrt_profile.argtypes = [ctypes.c_char_p]
            lib.axon_stop_nrt_profile.restype = ctypes.c_int64

            @contextlib.contextmanager
            def _hook(output_dir, device_ids):
                import jax

                jax.devices()
                if device_ids:
                    ids = (ctypes.c_int64 * len(device_ids))(*device_ids)
                    rc = lib.axon_start_nrt_profile(ids, len(device_ids))
                else:
                    rc = lib.axon_start_nrt_profile(None, 0)
                if rc != 0:
                    raise RuntimeError(f"axon_start_nrt_profile rc={rc}")
                try:
                    yield
                finally:
                    n = lib.axon_stop_nrt_profile(str(output_dir).encode())
                    print(f"profile: {n} file(s) -> {output_dir}", file=sys.stderr)

            hook = _hook

    mod = types.ModuleType("antenv.axon_hooks")
    mod.get_axon_ntff_profile_hook = lambda: hook
    mod.set_axon_ntff_profile_hook = lambda h: None
    sys.modules["antenv.axon_hooks"] = mod


def _patch_tile_drain():
    """walrus caps sync-waits at 1 per Drain; TileContext's tail drain can
    carry one wait per engine/DMA-queue used. Split them across drains."""
    import concourse.tile as tile
    import concourse.mybir as mybir
    from concourse.vector_clock import ScopedClock

    if getattr(tile.TileContext, "_drain_patch", False):
        return

    def _drain_and_barrier(self, tick_clock, wait_clock):
        nc = self.nc
        drain_inst = nc.sync.drain()
        wait_clock.add_sem_waits(
            drain_inst.ins, ScopedClock({None: tick_clock.global_clock})
        )
        inst = drain_inst.ins
        si = inst.sync_info
        waits = list(si.on_wait) if si and si.on_wait else []
        if len(waits) > 1:
            inst.sync_info = mybir.SyncInfo(
                on_wait=waits[:1], on_update=list(si.on_update or [])
            )
            for i in range(1, len(waits)):
                extra = nc.sync.drain()
                extra.ins.sync_info = mybir.SyncInfo(
                    on_wait=waits[i : i + 1], on_update=[]
                )
        nc.all_engine_barrier()
        assert self.sems is not None
        popped = nc._tile_sem_poison_stack.pop()
        assert popped is self._sem_poison
        nc.clear_and_free_semaphores(list(self.sems.allocated().values()))
        nc.all_engine_barrier()

    tile.TileContext._drain_and_barrier = _drain_and_barrier
    tile.TileContext._drain_patch = True


def _split_excess_waits(nc, limit=1):
    """This walrus build rejects instructions carrying more than one sync
    wait. Move excess waits onto same-engine NoOp carriers inserted just
    before the instruction in its block (engine streams preserve block
    order, so the waits still execute first)."""
    import concourse.mybir as mybir
    import concourse.bass_isa as bass_isa

    nop_op = nc.isa.Opcode.NEURON_ISA_TPB_OPCODE_NOP
    nop_instr, nop_fixups = bass_isa.isa_struct(nc.isa, nop_op, {})

    n_id = [0]
    for f in nc.m.functions:
        for bb in f.blocks:
            new_insts = []
            for inst in bb.instructions:
                si = getattr(inst, "sync_info", None)
                if si and si.on_wait and len(si.on_wait) > limit:
                    waits = list(si.on_wait)
                    head, keep = waits[:-limit], waits[-limit:]
                    for w in head:
                        if str(inst.engine) == "EngineType.SP":
                            nop = mybir.InstDrain(name=f"I-waitnop-{n_id[0]}")
                        else:
                            # ENGINE_NOP: carries the wait without a pipe flush
                            nop = mybir.InstISA(
                                name=f"I-waitnop-{n_id[0]}",
                                isa_opcode=nop_op.value,
                                engine=inst.engine,
                                instr=nop_instr,
                                op_name="NOP",
                                ins=[],
                                outs=[],
                                ant_dict={},
                                verify=True,
                                ant_isa_is_sequencer_only=True,
                                ant_sbuf_fixups=nop_fixups or None,
                            )
                        n_id[0] += 1
                        nop.engine = inst.engine
                        nop.sync_info = mybir.SyncInfo(on_wait=[w], on_update=[])
                        new_insts.append(nop)
                    inst.sync_info = mybir.SyncInfo(
                        on_wait=keep, on_update=list(si.on_update or [])
                    )
                new_insts.append(inst)
            if len(new_insts) != len(bb.instructions):
                bb.instructions[:] = new_insts


def _act_raw(nc, mybir, out, in_, func, bias=0.0, scale=1.0, accum_out=None):
    """InstActivation without the wrapper's Reciprocal/Rsqrt ban (measured
    LUT max rel err ~1.2e-5 on our value range). scale may be a [P,1] AP."""
    eng = nc.scalar
    ins = [eng.lower_ap(in_),
           mybir.ImmediateValue(dtype=mybir.dt.float32, value=float(bias))]
    if hasattr(scale, "space"):
        ins.append(eng.lower_ap(scale))
    else:
        ins.append(mybir.ImmediateValue(dtype=mybir.dt.float32, value=float(scale)))
    ins.append(mybir.ImmediateValue(dtype=mybir.dt.float32, value=0.0))
    outs = [eng.lower_ap(out)]
    if accum_out is not None:
        outs.append(eng.lower_ap(accum_out))
    return eng.add_instruction(mybir.InstActivation(
        name=nc.get_next_instruction_name(), func=func, ins=ins, outs=outs,
    ))


def _build():
    import concourse.bass as bass
    import concourse.mybir as mybir
    import concourse.tile as tile

    _patch_tile_drain()
    dt = mybir.dt
    AF = mybir.ActivationFunctionType
    OP = mybir.AluOpType
    X = mybir.AxisListType.X

    nc = bass.Bass(target_bir_lowering=False)

    prdT = nc.declare_dram_parameter("prdT", [D, RPC], dt.float32, isOutput=False)
    detT_hi = nc.declare_dram_parameter("detT_hi", [D, ND], dt.bfloat16, isOutput=False)
    wm1 = nc.declare_dram_parameter("wm1", [RPC, ND], dt.float8e4, isOutput=False)
    ident = nc.declare_dram_parameter("ident", [128, 128], dt.float32, isOutput=False)
    out1 = nc.declare_dram_parameter("out1", [RPC, ND], dt.float32, isOutput=True)
    ext = nc.declare_dram_parameter("ext", [RPC, ND + 1], dt.float32, isOutput=True)

    DCH = D // 128  # 4 contraction chunks

    with tile.TileContext(nc) as tc:
        with (
            tc.tile_pool(name="persist", bufs=1) as persist,
            tc.tile_pool(name="dettmp", bufs=1) as dettmp,
            tc.tile_pool(name="small", bufs=1) as small,
            tc.tile_pool(name="stream", bufs=2) as stream,
            tc.tile_pool(name="rowbuf", bufs=2) as rowbuf,
            tc.tile_pool(name="psum", bufs=1, space="PSUM") as psum,
        ):
            # ---------------- constants -------------------------------------
            ident_sb = small.tile([128, 128], dt.float32)
            nc.sync.dma_start(out=ident_sb, in_=ident[:, :])
            ones_bf = small.tile([128, 128], dt.bfloat16)
            nc.vector.memset(ones_bf, 1.0)

            # ---------------- prd-side prep ---------------------------------
            # sumsq per prd row i in FREE layout: square prdT chunks (bf16),
            # ones-matmul accumulates over d -> psum[*, i] (bcast over
            # partitions). Then r_i = sqrt(1/ss) and 99*n_i = sqrt(9801*ss).
            pr_ss_ps = psum.tile([128, RPC], dt.float32, tag="p", bufs=7)
            prdT_st = []
            for ch in range(DCH):
                pt_sb = stream.tile([128, RPC], dt.float32, tag=f"ptc{ch}", bufs=1)
                nc.sync.dma_start(out=pt_sb, in_=prdT[ch * 128 : (ch + 1) * 128, :])
                prdT_st.append(pt_sb)
                psq = stream.tile([128, RPC], dt.bfloat16, tag="psq", bufs=1)
                nc.vector.tensor_tensor(out=psq, in0=pt_sb, in1=pt_sb, op=OP.mult)
                nc.tensor.matmul(
                    pr_ss_ps, ones_bf[:, :], psq[:, :],
                    start=(ch == 0), stop=(ch == DCH - 1),
                )
            r_free = small.tile([128, RPC], dt.float32)   # 1/||row i||, free axis
            _act_raw(nc, mybir, out=r_free, in_=pr_ss_ps, func=AF.Rsqrt)
            # 99*I split as 96*I + 3*I (both exact in fp8e4) for the fp8 W inject
            diag96 = small.tile([128, 128], dt.float8e4)
            nc.vector.tensor_scalar(
                out=diag96, in0=ident_sb, scalar1=96.0, scalar2=None, op0=OP.mult
            )
            diag3 = small.tile([128, 128], dt.float8e4)
            nc.vector.tensor_scalar(
                out=diag3, in0=ident_sb, scalar1=3.0, scalar2=None, op0=OP.mult
            )
            # scale prdT by r_i (free-broadcast)
            prdT_r = []
            for ch in range(DCH):
                pt_r = persist.tile([128, RPC], dt.bfloat16, tag=f"prdT{ch}")
                nc.vector.tensor_tensor(
                    out=pt_r, in0=prdT_st[ch], in1=r_free, op=OP.mult
                )
                prdT_r.append(pt_r)

            # ---------------- det-side prep ---------------------------------
            # bf16 det: DMA straight into the matmul operand tiles, square in
            # 2x mode for norms, then scale by c in place (sq reads serialize
            # before the in-place scale via subregion deps).
            det_r = []
            for ch in range(DCH):
                dr = persist.tile([128, ND], dt.bfloat16, tag=f"detr{ch}")
                det_r.append(dr)
            c_rcp = dettmp.tile([128, ND], dt.bfloat16, tag="crcp")
            HW = ND // 2
            for half in range(2):
                hsl = slice(half * HW, (half + 1) * HW)
                dsqs = []
                for ch in range(DCH):
                    rows = slice(ch * 128, (ch + 1) * 128)
                    nc.sync.dma_start(out=det_r[ch][:, hsl], in_=detT_hi[rows, hsl])
                    dsq = dettmp.tile([128, HW], dt.bfloat16, tag=f"dsq{ch}", bufs=1)
                    nc.vector.tensor_tensor(
                        out=dsq, in0=det_r[ch][:, hsl],
                        in1=det_r[ch][:, hsl], op=OP.mult,
                    )
                    dsqs.append(dsq)
                for j in range(HW // 512):
                    jb = half * (HW // 512) + j
                    sl = slice(jb * 512, (jb + 1) * 512)
                    c_sq_ps = psum.tile([128, 512], dt.float32, tag="csq", bufs=1)
                    for ch in range(DCH):
                        nc.tensor.matmul(
                            c_sq_ps,
                            ones_bf[:, :],
                            dsqs[ch][:, j * 512 : (j + 1) * 512],
                            start=(ch == 0),
                            stop=(ch == DCH - 1),
                        )
                    _act_raw(nc, mybir, out=c_rcp[:, sl], in_=c_sq_ps,
                             func=AF.Rsqrt)
                    for ch in range(DCH):
                        nc.vector.tensor_tensor(
                            out=det_r[ch][:, sl],
                            in0=det_r[ch][:, sl],
                            in1=c_rcp[:, sl],
                            op=OP.mult,
                        )

            # ---------------- main loop -------------------------------------
            for t in range(RT):
                ex_sb = rowbuf.tile([128, ND + 1], dt.float32, tag="ex", bufs=3)
                o1_sb = stream.tile([128, ND], dt.float32, tag="o1full", bufs=2)
                srelu = stream.tile([128, JB], dt.float32, tag="srelu")
                hsum = stream.tile([128, JB], dt.float32, tag="hsum")
                for jb in range(JB):
                    w_sb = stream.tile([128, 512], dt.float8e4, tag="w", bufs=6)
                    nc.gpsimd.dma_start(
                        out=w_sb,
                        in_=wm1[t * 128 : (t + 1) * 128, jb * 512 : (jb + 1) * 512],
                    )
                    p = psum.tile([128, 512], dt.float32, tag="p", bufs=7)
                    for ch in range(DCH):
                        nc.tensor.matmul(
                            p,
                            prdT_r[ch][:, t * 128 : (t + 1) * 128],
                            det_r[ch][:, jb * 512 : (jb + 1) * 512],
                            start=(ch == 0),
                            stop=False,
                        )
                    nc.tensor.matmul(
                        p, diag96[:, :], w_sb[:, :],
                        start=False, stop=False, skip_group_check=True,
                    )
                    nc.tensor.matmul(
                        p, diag3[:, :], w_sb[:, :],
                        start=False, stop=True, skip_group_check=True,
                    )
                    # out1 = relu(psum); accum-add -> any-positive detector
                    nc.vector.tensor_scalar(
                        out=o1_sb[:, jb * 512 : (jb + 1) * 512],
                        in0=p, scalar1=0.0, scalar2=0.0,
                        op0=OP.max, op1=OP.add,
                        accum_out=srelu[:, jb : jb + 1],
                    )
                    # half = 1/(1 - masked); accum row sums
                    _act_raw(nc, mybir,
                             out=ex_sb[:, jb * 512 : (jb + 1) * 512],
                             in_=p, func=AF.Reciprocal,
                             bias=1.0, scale=-1.0,
                             accum_out=hsum[:, jb : jb + 1])
                    if jb == JB // 2 - 1:
                        nc.scalar.dma_start(
                            out=out1[t * 128 : (t + 1) * 128, : ND // 2],
                            in_=o1_sb[:, : ND // 2],
                        )
                # row stats
                rowsum = stream.tile([128, 1], dt.float32, tag="rowsum")
                nc.vector.tensor_reduce(out=rowsum, in_=hsum, axis=X, op=OP.add)
                anyrel = stream.tile([128, 1], dt.float32, tag="anyrel")
                nc.vector.tensor_reduce(out=anyrel, in_=srelu, axis=X, op=OP.add)
                rs_inv = stream.tile([128, 1], dt.float32, tag="rsinv")
                nc.vector.reciprocal(out=rs_inv, in_=rowsum)
                g = stream.tile([128, 1], dt.float32, tag="g")
                nc.vector.tensor_scalar(
                    out=g, in0=anyrel, scalar1=0.0, scalar2=None, op0=OP.is_gt
                )
                s_row = stream.tile([128, 1], dt.float32, tag="srow")
                nc.vector.tensor_tensor(out=s_row, in0=rs_inv, in1=g, op=OP.mult)
                supple = stream.tile([128, 1], dt.float32, tag="supple")
                nc.vector.tensor_scalar(
                    out=supple, in0=g, scalar1=-1.0, scalar2=1.0,
                    op0=OP.mult, op1=OP.add,
                )
                nc.scalar.dma_start(
                    out=out1[t * 128 : (t + 1) * 128, ND // 2 :],
                    in_=o1_sb[:, ND // 2 :],
                )
                for jb in range(JB):
                    nc.vector.tensor_scalar(
                        out=ex_sb[:, jb * 512 : (jb + 1) * 512],
                        in0=ex_sb[:, jb * 512 : (jb + 1) * 512],
                        scalar1=s_row[:, :], scalar2=None, op0=OP.mult,
                    )
                nc.vector.tensor_copy(ex_sb[:, ND : ND + 1], supple)
                nc.scalar.dma_start(
                    out=ext[t * 128 : (t + 1) * 128, :], in_=ex_sb
                )
    _split_excess_waits(nc)
    return nc


def _get_nc():
    if "nc" not in _cache:
        _cache["nc"] = _build()
    return _cache["nc"]


def kernel(embeddings, W, len_prd=LP, head=1):
    from concourse.bass_utils import run_bass_kernel_spmd

    embeddings = np.asarray(embeddings, dtype=np.float32)
    W = np.asarray(W)
    assert int(len_prd) == LP and int(head) == 1
    assert embeddings.shape == (N_TOT, D) and W.shape == (N_TOT, N_TOT)

    nc = _get_nc()

    detT = np.ascontiguousarray(embeddings[LP:].T)           # [D, ND] f32
    detT_hi = detT.astype(ml_dtypes.bfloat16)
    # (W - 1) as bf16 without a float conversion pass: 0 -> -1.0, 1 -> 0.0
    Wb = W[:LP, LP:]
    wm1_bits = np.where(Wb == 0, np.uint8(0xB8), np.uint8(0)).astype(np.uint8)
    wm1 = wm1_bits.view(ml_dtypes.float8_e4m3)               # [LP, ND] fp8
    ident = np.eye(128, dtype=np.float32)

    in_maps = []
    for c in range(C):
        rows = slice(c * RPC, (c + 1) * RPC)
        in_maps.append(
            {
                "prdT": np.ascontiguousarray(embeddings[rows].T),
                "detT_hi": detT_hi,
                "wm1": np.ascontiguousarray(wm1[rows]),
                "ident": ident,
            }
        )

    trace = bool(os.environ.get("BASS_KERNEL_TRACE"))
    if trace:
        _install_ntff_hook_shim()
    res = run_bass_kernel_spmd(
        nc, in_maps, core_ids=list(range(C)), trace=trace
    )
    if trace:
        _cache["exec_time_ns"] = res.exec_time_ns
        _cache["last_results"] = res
        print(f"HW exec time: {res.exec_time_ns} ns", file=sys.stderr)

    out1 = np.concatenate([r["out1"] for r in res.results], axis=0)
    extended = np.concatenate([r["ext"] for r in res.results], axis=0)
    return out1, extended


# revision 47
# speedup vs baseline: 1.0513x; 1.0513x over previous
"""Trainium2 Bass kernel for nn_ClassifyLayer (masked cosine-similarity classify layer).

Reference math (head==1, len_prd=4096, N=8192, D=512):
  Wb     = W[:4096, 4096:]                     (0/1 ints)
  mask   = -99 * (1 - Wb)
  ne     = embeddings / ||embeddings||_row
  sim    = ne[:4096] @ ne[4096:].T             [4096, 4096]
  masked = sim + mask
  out1   = clamp01(relu(masked))
  half   = 1 / (1 - masked)
  ext    = [half / rowsum(half) (zeroed where no positive in row), supple_flag]

Sharding: row-parallel over prd across 8 cores (512 rows/core); the det block
(embeddings[4096:], transposed host-side) is replicated to every core.

On-chip per core:
  PSUM = sum_d eprdT[d,i] * (edetT[d,j]*c_j)  (4x float32r matmuls, K=128 each)
       + sum_k diag(99*n_i)[k,i] * (W-1)[k,j] (1x bf16 matmul)  == masked / r_i
  ACT:  out1 = Relu(r_i * PSUM)           (+accum -> "any positive" flag)
        t    = 1 - r_i * PSUM             (Copy, scale=-r_i, bias=1)
  DVE:  half = reciprocal_approx_fast(t); rowsum accum; ext = half * s_row
where n_i = ||prd row i||, r_i = 1/n_i, c_j = 1/||det row j||.
"""

import os
import sys

sys.path.insert(0, "/opt/trn_rl_repo")

import numpy as np
import ml_dtypes

N_TOT, D, LP = 8192, 512, 4096
ND = N_TOT - LP          # 4096 det rows
C = 8                    # cores
RPC = LP // C            # 512 prd rows per core
RT = RPC // 128          # 4 row-tiles per core
JB = ND // 512           # 8 col-blocks of 512
NEG = 99.0

_cache = {}


def _install_ntff_hook_shim():
    """This image's `antenv` lacks `axon_hooks`; provide it so
    run_bass_kernel_spmd(trace=True) can capture NTFF profiles via the
    axon PJRT .so (same ctypes dance as trn_agent_boot)."""
    import types
    import ctypes
    import contextlib

    if "antenv.axon_hooks" in sys.modules:
        return
    so_path = "/opt/axon/libaxon_pjrt.so"
    hook = None
    if os.path.exists(so_path):
        lib = ctypes.CDLL(so_path)
        if hasattr(lib, "axon_start_nrt_profile"):
            lib.axon_start_nrt_profile.argtypes = [
                ctypes.POINTER(ctypes.c_int64),
                ctypes.c_size_t,
            ]
            lib.axon_start_nrt_profile.restype = ctypes.c_int64
            lib.axon_stop_nrt_profile.argtypes = [ctypes.c_char_p]
            lib.axon_stop_nrt_profile.restype = ctypes.c_int64

            @contextlib.contextmanager
            def _hook(output_dir, device_ids):
                import jax

                jax.devices()
                if device_ids:
                    ids = (ctypes.c_int64 * len(device_ids))(*device_ids)
                    rc = lib.axon_start_nrt_profile(ids, len(device_ids))
                else:
                    rc = lib.axon_start_nrt_profile(None, 0)
                if rc != 0:
                    raise RuntimeError(f"axon_start_nrt_profile rc={rc}")
                try:
                    yield
                finally:
                    n = lib.axon_stop_nrt_profile(str(output_dir).encode())
                    print(f"profile: {n} file(s) -> {output_dir}", file=sys.stderr)

            hook = _hook

    mod = types.ModuleType("antenv.axon_hooks")
    mod.get_axon_ntff_profile_hook = lambda: hook
    mod.set_axon_ntff_profile_hook = lambda h: None
    sys.modules["antenv.axon_hooks"] = mod


def _patch_tile_drain():
    """walrus caps sync-waits at 1 per Drain; TileContext's tail drain can
    carry one wait per engine/DMA-queue used. Split them across drains."""
    import concourse.tile as tile
    import concourse.mybir as mybir
    from concourse.vector_clock import ScopedClock

    if getattr(tile.TileContext, "_drain_patch", False):
        return

    def _drain_and_barrier(self, tick_clock, wait_clock):
        nc = self.nc
        drain_inst = nc.sync.drain()
        wait_clock.add_sem_waits(
            drain_inst.ins, ScopedClock({None: tick_clock.global_clock})
        )
        inst = drain_inst.ins
        si = inst.sync_info
        waits = list(si.on_wait) if si and si.on_wait else []
        if len(waits) > 1:
            inst.sync_info = mybir.SyncInfo(
                on_wait=waits[:1], on_update=list(si.on_update or [])
            )
            for i in range(1, len(waits)):
                extra = nc.sync.drain()
                extra.ins.sync_info = mybir.SyncInfo(
                    on_wait=waits[i : i + 1], on_update=[]
                )
        nc.all_engine_barrier()
        assert self.sems is not None
        popped = nc._tile_sem_poison_stack.pop()
        assert popped is self._sem_poison
        nc.clear_and_free_semaphores(list(self.sems.allocated().values()))
        nc.all_engine_barrier()

    tile.TileContext._drain_and_barrier = _drain_and_barrier
    tile.TileContext._drain_patch = True


def _split_excess_waits(nc, limit=1):
    """This walrus build rejects instructions carrying more than one sync
    wait. Move excess waits onto same-engine NoOp carriers inserted just
    before the instruction in its block (engine streams preserve block
    order, so the waits still execute first)."""
    import concourse.mybir as mybir
    import concourse.bass_isa as bass_isa

    nop_op = nc.isa.Opcode.NEURON_ISA_TPB_OPCODE_NOP
    nop_instr, nop_fixups = bass_isa.isa_struct(nc.isa, nop_op, {})

    n_id = [0]
    for f in nc.m.functions:
        for bb in f.blocks:
            new_insts = []
            for inst in bb.instructions:
                si = getattr(inst, "sync_info", None)
                if si and si.on_wait and len(si.on_wait) > limit:
                    waits = list(si.on_wait)
                    head, keep = waits[:-limit], waits[-limit:]
                    for w in head:
                        if str(inst.engine) == "EngineType.SP":
                            nop = mybir.InstDrain(name=f"I-waitnop-{n_id[0]}")
                        else:
                            # ENGINE_NOP: carries the wait without a pipe flush
                            nop = mybir.InstISA(
                                name=f"I-waitnop-{n_id[0]}",
                                isa_opcode=nop_op.value,
                                engine=inst.engine,
                                instr=nop_instr,
                                op_name="NOP",
                                ins=[],
                                outs=[],
                                ant_dict={},
                                verify=True,
                                ant_isa_is_sequencer_only=True,
                                ant_sbuf_fixups=nop_fixups or None,
                            )
                        n_id[0] += 1
                        nop.engine = inst.engine
                        nop.sync_info = mybir.SyncInfo(on_wait=[w], on_update=[])
                        new_insts.append(nop)
                    inst.sync_info = mybir.SyncInfo(
                        on_wait=keep, on_update=list(si.on_update or [])
                    )
                new_insts.append(inst)
            if len(new_insts) != len(bb.instructions):
                bb.instructions[:] = new_insts


def _act_raw(nc, mybir, out, in_, func, bias=0.0, scale=1.0, accum_out=None):
    """InstActivation without the wrapper's Reciprocal/Rsqrt ban (measured
    LUT max rel err ~1.2e-5 on our value range). scale may be a [P,1] AP."""
    eng = nc.scalar
    ins = [eng.lower_ap(in_),
           mybir.ImmediateValue(dtype=mybir.dt.float32, value=float(bias))]
    if hasattr(scale, "space"):
        ins.append(eng.lower_ap(scale))
    else:
        ins.append(mybir.ImmediateValue(dtype=mybir.dt.float32, value=float(scale)))
    ins.append(mybir.ImmediateValue(dtype=mybir.dt.float32, value=0.0))
    outs = [eng.lower_ap(out)]
    if accum_out is not None:
        outs.append(eng.lower_ap(accum_out))
    return eng.add_instruction(mybir.InstActivation(
        name=nc.get_next_instruction_name(), func=func, ins=ins, outs=outs,
    ))


def _build():
    import concourse.bass as bass
    import concourse.mybir as mybir
    import concourse.tile as tile

    _patch_tile_drain()
    dt = mybir.dt
    AF = mybir.ActivationFunctionType
    OP = mybir.AluOpType
    X = mybir.AxisListType.X

    nc = bass.Bass(target_bir_lowering=False)

    prdT = nc.declare_dram_parameter("prdT", [D, RPC], dt.float32, isOutput=False)
    detT_hi = nc.declare_dram_parameter("detT_hi", [D, ND], dt.bfloat16, isOutput=False)
    wm1 = nc.declare_dram_parameter("wm1", [RPC, ND], dt.float8e4, isOutput=False)
    ident = nc.declare_dram_parameter("ident", [128, 128], dt.float32, isOutput=False)
    out1 = nc.declare_dram_parameter("out1", [RPC, ND], dt.float32, isOutput=True)
    ext = nc.declare_dram_parameter("ext", [RPC, ND + 1], dt.float32, isOutput=True)

    DCH = D // 128  # 4 contraction chunks

    with tile.TileContext(nc) as tc:
        with (
            tc.tile_pool(name="persist", bufs=1) as persist,
            tc.tile_pool(name="dettmp", bufs=1) as dettmp,
            tc.tile_pool(name="small", bufs=1) as small,
            tc.tile_pool(name="stream", bufs=2) as stream,
            tc.tile_pool(name="rowbuf", bufs=2) as rowbuf,
            tc.tile_pool(name="psum", bufs=1, space="PSUM") as psum,
        ):
            # ---------------- constants -------------------------------------
            ident_sb = small.tile([128, 128], dt.float32)
            nc.sync.dma_start(out=ident_sb, in_=ident[:, :])
            ones_bf = small.tile([128, 128], dt.bfloat16)
            nc.vector.memset(ones_bf, 1.0)

            # ---------------- prd-side prep ---------------------------------
            # sumsq per prd row i in FREE layout: square prdT chunks (bf16),
            # ones-matmul accumulates over d -> psum[*, i] (bcast over
            # partitions). Then r_i = sqrt(1/ss) and 99*n_i = sqrt(9801*ss).
            pr_ss_ps = psum.tile([128, RPC], dt.float32, tag="p", bufs=7)
            prdT_st = []
            for ch in range(DCH):
                pt_sb = stream.tile([128, RPC], dt.float32, tag=f"ptc{ch}", bufs=1)
                nc.sync.dma_start(out=pt_sb, in_=prdT[ch * 128 : (ch + 1) * 128, :])
                prdT_st.append(pt_sb)
                psq = stream.tile([128, RPC], dt.bfloat16, tag="psq", bufs=1)
                nc.vector.tensor_tensor(out=psq, in0=pt_sb, in1=pt_sb, op=OP.mult)
                nc.tensor.matmul(
                    pr_ss_ps, ones_bf[:, :], psq[:, :],
                    start=(ch == 0), stop=(ch == DCH - 1),
                )
            r_free = small.tile([128, RPC], dt.float32)   # 1/||row i||, free axis
            _act_raw(nc, mybir, out=r_free, in_=pr_ss_ps, func=AF.Rsqrt)
            # 99*I split as 96*I + 3*I (both exact in fp8e4) for the fp8 W inject
            diag96 = small.tile([128, 128], dt.float8e4)
            nc.vector.tensor_scalar(
                out=diag96, in0=ident_sb, scalar1=96.0, scalar2=None, op0=OP.mult
            )
            diag3 = small.tile([128, 128], dt.float8e4)
            nc.vector.tensor_scalar(
                out=diag3, in0=ident_sb, scalar1=3.0, scalar2=None, op0=OP.mult
            )
            # scale prdT by r_i (free-broadcast)
            prdT_r = []
            for ch in range(DCH):
                pt_r = persist.tile([128, RPC], dt.bfloat16, tag=f"prdT{ch}")
                nc.vector.tensor_tensor(
                    out=pt_r, in0=prdT_st[ch], in1=r_free, op=OP.mult
                )
                prdT_r.append(pt_r)

            # ---------------- det-side prep ---------------------------------
            # bf16 det: DMA straight into the matmul operand tiles, square in
            # 2x mode for norms, then scale by c in place (sq reads serialize
            # before the in-place scale via subregion deps).
            det_r = []
            for ch in range(DCH):
                dr = persist.tile([128, ND], dt.bfloat16, tag=f"detr{ch}")
                det_r.append(dr)
            c_rcp = dettmp.tile([128, ND], dt.bfloat16, tag="crcp")
            HW = ND // 2
            for half in range(2):
                hsl = slice(half * HW, (half + 1) * HW)
                dsqs = []
                for ch in range(DCH):
                    rows = slice(ch * 128, (ch + 1) * 128)
                    nc.sync.dma_start(out=det_r[ch][:, hsl], in_=detT_hi[rows, hsl])
                    dsq = dettmp.tile([128, HW], dt.bfloat16, tag=f"dsq{ch}", bufs=1)
                    nc.vector.tensor_tensor(
                        out=dsq, in0=det_r[ch][:, hsl],
                        in1=det_r[ch][:, hsl], op=OP.mult,
                    )
                    dsqs.append(dsq)
                for j in range(HW // 512):
                    jb = half * (HW // 512) + j
                    sl = slice(jb * 512, (jb + 1) * 512)
                    c_sq_ps = psum.tile([128, 512], dt.float32, tag="csq", bufs=1)
                    for ch in range(DCH):
                        nc.tensor.matmul(
                            c_sq_ps,
                            ones_bf[:, :],
                            dsqs[ch][:, j * 512 : (j + 1) * 512],
                            start=(ch == 0),
                            stop=(ch == DCH - 1),
                        )
                    _act_raw(nc, mybir, out=c_rcp[:, sl], in_=c_sq_ps,
                             func=AF.Rsqrt)
                    for ch in range(DCH):
                        nc.vector.tensor_tensor(
                            out=det_r[ch][:, sl],
                            in0=det_r[ch][:, sl],
                            in1=c_rcp[:, sl],
                            op=OP.mult,
                        )

            # ---------------- main loop -------------------------------------
            # Row-tile pairs, left column-half first for both, so the PE
            # never stalls on the right det half while it still streams in.
            for t0 in range(0, RT, 2):
                pair = [t0, t0 + 1]
                ex_t, o1_t, srelu_t, hsum_t = {}, {}, {}, {}
                for t in pair:
                    ex_t[t] = rowbuf.tile([128, ND + 1], dt.float32, tag="ex", bufs=3, name=f"ex{t}")
                    o1_t[t] = stream.tile([128, ND], dt.float32, tag="o1full", bufs=2, name=f"o1{t}")
                    srelu_t[t] = stream.tile([128, JB], dt.float32, tag="srelu", bufs=3, name=f"sr{t}")
                    hsum_t[t] = stream.tile([128, JB], dt.float32, tag="hsum", bufs=3, name=f"hs{t}")
                for phase in range(2):
                    jbs = range(phase * (JB // 2), (phase + 1) * (JB // 2))
                    for t in pair:
                        ex_sb, o1_sb = ex_t[t], o1_t[t]
                        srelu, hsum = srelu_t[t], hsum_t[t]
                        for jb in jbs:
                            w_sb = stream.tile([128, 512], dt.float8e4, tag="w", bufs=6)
                            nc.gpsimd.dma_start(
                                out=w_sb,
                                in_=wm1[t * 128 : (t + 1) * 128,
                                        jb * 512 : (jb + 1) * 512],
                            )
                            p = psum.tile([128, 512], dt.float32, tag="p", bufs=7)
                            for ch in range(DCH):
                                nc.tensor.matmul(
                                    p,
                                    prdT_r[ch][:, t * 128 : (t + 1) * 128],
                                    det_r[ch][:, jb * 512 : (jb + 1) * 512],
                                    start=(ch == 0),
                                    stop=False,
                                )
                            nc.tensor.matmul(
                                p, diag96[:, :], w_sb[:, :],
                                start=False, stop=False, skip_group_check=True,
                            )
                            nc.tensor.matmul(
                                p, diag3[:, :], w_sb[:, :],
                                start=False, stop=True, skip_group_check=True,
                            )
                            nc.vector.tensor_scalar(
                                out=o1_sb[:, jb * 512 : (jb + 1) * 512],
                                in0=p, scalar1=0.0, scalar2=0.0,
                                op0=OP.max, op1=OP.add,
                                accum_out=srelu[:, jb : jb + 1],
                            )
                            _act_raw(nc, mybir,
                                     out=ex_sb[:, jb * 512 : (jb + 1) * 512],
                                     in_=p, func=AF.Reciprocal,
                                     bias=1.0, scale=-1.0,
                                     accum_out=hsum[:, jb : jb + 1])
                        nc.scalar.dma_start(
                            out=out1[t * 128 : (t + 1) * 128,
                                     phase * (ND // 2) : (phase + 1) * (ND // 2)],
                            in_=o1_sb[:, phase * (ND // 2) : (phase + 1) * (ND // 2)],
                        )
                for t in pair:
                    ex_sb = ex_t[t]
                    srelu, hsum = srelu_t[t], hsum_t[t]
                    rowsum = stream.tile([128, 1], dt.float32, tag="rowsum")
                    nc.vector.tensor_reduce(out=rowsum, in_=hsum, axis=X, op=OP.add)
                    anyrel = stream.tile([128, 1], dt.float32, tag="anyrel")
                    nc.vector.tensor_reduce(out=anyrel, in_=srelu, axis=X, op=OP.add)
                    rs_inv = stream.tile([128, 1], dt.float32, tag="rsinv")
                    nc.vector.reciprocal(out=rs_inv, in_=rowsum)
                    g = stream.tile([128, 1], dt.float32, tag="g")
                    nc.vector.tensor_scalar(
                        out=g, in0=anyrel, scalar1=0.0, scalar2=None, op0=OP.is_gt
                    )
                    s_row = stream.tile([128, 1], dt.float32, tag="srow")
                    nc.vector.tensor_tensor(out=s_row, in0=rs_inv, in1=g, op=OP.mult)
                    supple = stream.tile([128, 1], dt.float32, tag="supple")
                    nc.vector.tensor_scalar(
                        out=supple, in0=g, scalar1=-1.0, scalar2=1.0,
                        op0=OP.mult, op1=OP.add,
                    )
                    for jb in range(JB):
                        nc.vector.tensor_scalar(
                            out=ex_sb[:, jb * 512 : (jb + 1) * 512],
                            in0=ex_sb[:, jb * 512 : (jb + 1) * 512],
                            scalar1=s_row[:, :], scalar2=None, op0=OP.mult,
                        )
                    nc.vector.tensor_copy(ex_sb[:, ND : ND + 1], supple)
                    nc.scalar.dma_start(
                        out=ext[t * 128 : (t + 1) * 128, :], in_=ex_sb
                    )
    _split_excess_waits(nc)
    return nc


def _get_nc():
    if "nc" not in _cache:
        _cache["nc"] = _build()
    return _cache["nc"]


def kernel(embeddings, W, len_prd=LP, head=1):
    from concourse.bass_utils import run_bass_kernel_spmd

    embeddings = np.asarray(embeddings, dtype=np.float32)
    W = np.asarray(W)
    assert int(len_prd) == LP and int(head) == 1
    assert embeddings.shape == (N_TOT, D) and W.shape == (N_TOT, N_TOT)

    nc = _get_nc()

    detT = np.ascontiguousarray(embeddings[LP:].T)           # [D, ND] f32
    detT_hi = detT.astype(ml_dtypes.bfloat16)
    # (W - 1) as bf16 without a float conversion pass: 0 -> -1.0, 1 -> 0.0
    Wb = W[:LP, LP:]
    wm1_bits = np.where(Wb == 0, np.uint8(0xB8), np.uint8(0)).astype(np.uint8)
    wm1 = wm1_bits.view(ml_dtypes.float8_e4m3)               # [LP, ND] fp8
    ident = np.eye(128, dtype=np.float32)

    in_maps = []
    for c in range(C):
        rows = slice(c * RPC, (c + 1) * RPC)
        in_maps.append(
            {
                "prdT": np.ascontiguousarray(embeddings[rows].T),
                "detT_hi": detT_hi,
                "wm1": np.ascontiguousarray(wm1[rows]),
                "ident": ident,
            }
        )

    trace = bool(os.environ.get("BASS_KERNEL_TRACE"))
    if trace:
        _install_ntff_hook_shim()
    res = run_bass_kernel_spmd(
        nc, in_maps, core_ids=list(range(C)), trace=trace
    )
    if trace:
        _cache["exec_time_ns"] = res.exec_time_ns
        _cache["last_results"] = res
        print(f"HW exec time: {res.exec_time_ns} ns", file=sys.stderr)

    out1 = np.concatenate([r["out1"] for r in res.results], axis=0)
    extended = np.concatenate([r["ext"] for r in res.results], axis=0)
    return out1, extended


# revision 48
# speedup vs baseline: 1.0549x; 1.0034x over previous
"""Trainium2 Bass kernel for nn_ClassifyLayer (masked cosine-similarity classify layer).

Reference math (head==1, len_prd=4096, N=8192, D=512):
  Wb     = W[:4096, 4096:]                     (0/1 ints)
  mask   = -99 * (1 - Wb)
  ne     = embeddings / ||embeddings||_row
  sim    = ne[:4096] @ ne[4096:].T             [4096, 4096]
  masked = sim + mask
  out1   = clamp01(relu(masked))
  half   = 1 / (1 - masked)
  ext    = [half / rowsum(half) (zeroed where no positive in row), supple_flag]

Sharding: row-parallel over prd across 8 cores (512 rows/core); the det block
(embeddings[4096:], transposed host-side) is replicated to every core.

On-chip per core:
  PSUM = sum_d eprdT[d,i] * (edetT[d,j]*c_j)  (4x float32r matmuls, K=128 each)
       + sum_k diag(99*n_i)[k,i] * (W-1)[k,j] (1x bf16 matmul)  == masked / r_i
  ACT:  out1 = Relu(r_i * PSUM)           (+accum -> "any positive" flag)
        t    = 1 - r_i * PSUM             (Copy, scale=-r_i, bias=1)
  DVE:  half = reciprocal_approx_fast(t); rowsum accum; ext = half * s_row
where n_i = ||prd row i||, r_i = 1/n_i, c_j = 1/||det row j||.
"""

import os
import sys

sys.path.insert(0, "/opt/trn_rl_repo")

import numpy as np
import ml_dtypes

N_TOT, D, LP = 8192, 512, 4096
ND = N_TOT - LP          # 4096 det rows
C = 8                    # cores
RPC = LP // C            # 512 prd rows per core
RT = RPC // 128          # 4 row-tiles per core
JB = ND // 512           # 8 col-blocks of 512
NEG = 99.0

_cache = {}


def _install_ntff_hook_shim():
    """This image's `antenv` lacks `axon_hooks`; provide it so
    run_bass_kernel_spmd(trace=True) can capture NTFF profiles via the
    axon PJRT .so (same ctypes dance as trn_agent_boot)."""
    import types
    import ctypes
    import contextlib

    if "antenv.axon_hooks" in sys.modules:
        return
    so_path = "/opt/axon/libaxon_pjrt.so"
    hook = None
    if os.path.exists(so_path):
        lib = ctypes.CDLL(so_path)
        if hasattr(lib, "axon_start_nrt_profile"):
            lib.axon_start_nrt_profile.argtypes = [
                ctypes.POINTER(ctypes.c_int64),
                ctypes.c_size_t,
            ]
            lib.axon_start_nrt_profile.restype = ctypes.c_int64
            lib.axon_stop_nrt_profile.argtypes = [ctypes.c_char_p]
            lib.axon_stop_nrt_profile.restype = ctypes.c_int64

            @contextlib.contextmanager
            def _hook(output_dir, device_ids):
                import jax

                jax.devices()
                if device_ids:
                    ids = (ctypes.c_int64 * len(device_ids))(*device_ids)
                    rc = lib.axon_start_nrt_profile(ids, len(device_ids))
                else:
                    rc = lib.axon_start_nrt_profile(None, 0)
                if rc != 0:
                    raise RuntimeError(f"axon_start_nrt_profile rc={rc}")
                try:
                    yield
                finally:
                    n = lib.axon_stop_nrt_profile(str(output_dir).encode())
                    print(f"profile: {n} file(s) -> {output_dir}", file=sys.stderr)

            hook = _hook

    mod = types.ModuleType("antenv.axon_hooks")
    mod.get_axon_ntff_profile_hook = lambda: hook
    mod.set_axon_ntff_profile_hook = lambda h: None
    sys.modules["antenv.axon_hooks"] = mod


def _patch_tile_drain():
    """walrus caps sync-waits at 1 per Drain; TileContext's tail drain can
    carry one wait per engine/DMA-queue used. Split them across drains."""
    import concourse.tile as tile
    import concourse.mybir as mybir
    from concourse.vector_clock import ScopedClock

    if getattr(tile.TileContext, "_drain_patch", False):
        return

    def _drain_and_barrier(self, tick_clock, wait_clock):
        nc = self.nc
        drain_inst = nc.sync.drain()
        wait_clock.add_sem_waits(
            drain_inst.ins, ScopedClock({None: tick_clock.global_clock})
        )
        inst = drain_inst.ins
        si = inst.sync_info
        waits = list(si.on_wait) if si and si.on_wait else []
        if len(waits) > 1:
            inst.sync_info = mybir.SyncInfo(
                on_wait=waits[:1], on_update=list(si.on_update or [])
            )
            for i in range(1, len(waits)):
                extra = nc.sync.drain()
                extra.ins.sync_info = mybir.SyncInfo(
                    on_wait=waits[i : i + 1], on_update=[]
                )
        nc.all_engine_barrier()
        assert self.sems is not None
        popped = nc._tile_sem_poison_stack.pop()
        assert popped is self._sem_poison
        nc.clear_and_free_semaphores(list(self.sems.allocated().values()))
        nc.all_engine_barrier()

    tile.TileContext._drain_and_barrier = _drain_and_barrier
    tile.TileContext._drain_patch = True


def _split_excess_waits(nc, limit=1):
    """This walrus build rejects instructions carrying more than one sync
    wait. Move excess waits onto same-engine NoOp carriers inserted just
    before the instruction in its block (engine streams preserve block
    order, so the waits still execute first)."""
    import concourse.mybir as mybir
    import concourse.bass_isa as bass_isa

    nop_op = nc.isa.Opcode.NEURON_ISA_TPB_OPCODE_NOP
    nop_instr, nop_fixups = bass_isa.isa_struct(nc.isa, nop_op, {})

    n_id = [0]
    for f in nc.m.functions:
        for bb in f.blocks:
            new_insts = []
            for inst in bb.instructions:
                si = getattr(inst, "sync_info", None)
                if si and si.on_wait and len(si.on_wait) > limit:
                    waits = list(si.on_wait)
                    head, keep = waits[:-limit], waits[-limit:]
                    for w in head:
                        if str(inst.engine) == "EngineType.SP":
                            nop = mybir.InstDrain(name=f"I-waitnop-{n_id[0]}")
                        else:
                            # ENGINE_NOP: carries the wait without a pipe flush
                            nop = mybir.InstISA(
                                name=f"I-waitnop-{n_id[0]}",
                                isa_opcode=nop_op.value,
                                engine=inst.engine,
                                instr=nop_instr,
                                op_name="NOP",
                                ins=[],
                                outs=[],
                                ant_dict={},
                                verify=True,
                                ant_isa_is_sequencer_only=True,
                                ant_sbuf_fixups=nop_fixups or None,
                            )
                        n_id[0] += 1
                        nop.engine = inst.engine
                        nop.sync_info = mybir.SyncInfo(on_wait=[w], on_update=[])
                        new_insts.append(nop)
                    inst.sync_info = mybir.SyncInfo(
                        on_wait=keep, on_update=list(si.on_update or [])
                    )
                new_insts.append(inst)
            if len(new_insts) != len(bb.instructions):
                bb.instructions[:] = new_insts


def _act_raw(nc, mybir, out, in_, func, bias=0.0, scale=1.0, accum_out=None):
    """InstActivation without the wrapper's Reciprocal/Rsqrt ban (measured
    LUT max rel err ~1.2e-5 on our value range). scale may be a [P,1] AP."""
    eng = nc.scalar
    ins = [eng.lower_ap(in_),
           mybir.ImmediateValue(dtype=mybir.dt.float32, value=float(bias))]
    if hasattr(scale, "space"):
        ins.append(eng.lower_ap(scale))
    else:
        ins.append(mybir.ImmediateValue(dtype=mybir.dt.float32, value=float(scale)))
    ins.append(mybir.ImmediateValue(dtype=mybir.dt.float32, value=0.0))
    outs = [eng.lower_ap(out)]
    if accum_out is not None:
        outs.append(eng.lower_ap(accum_out))
    return eng.add_instruction(mybir.InstActivation(
        name=nc.get_next_instruction_name(), func=func, ins=ins, outs=outs,
    ))


def _build():
    import concourse.bass as bass
    import concourse.mybir as mybir
    import concourse.tile as tile

    _patch_tile_drain()
    dt = mybir.dt
    AF = mybir.ActivationFunctionType
    OP = mybir.AluOpType
    X = mybir.AxisListType.X

    nc = bass.Bass(target_bir_lowering=False)

    prdT = nc.declare_dram_parameter("prdT", [D, RPC], dt.float32, isOutput=False)
    detT_hi = nc.declare_dram_parameter("detT_hi", [D, ND], dt.bfloat16, isOutput=False)
    wm1 = nc.declare_dram_parameter("wm1", [RPC, ND], dt.float8e4, isOutput=False)
    ident = nc.declare_dram_parameter("ident", [128, 128], dt.float32, isOutput=False)
    out1 = nc.declare_dram_parameter("out1", [RPC, ND], dt.float32, isOutput=True)
    ext = nc.declare_dram_parameter("ext", [RPC, ND + 1], dt.float32, isOutput=True)

    DCH = D // 128  # 4 contraction chunks

    with tile.TileContext(nc) as tc:
        with (
            tc.tile_pool(name="persist", bufs=1) as persist,
            tc.tile_pool(name="dettmp", bufs=1) as dettmp,
            tc.tile_pool(name="small", bufs=1) as small,
            tc.tile_pool(name="stream", bufs=2) as stream,
            tc.tile_pool(name="rowbuf", bufs=2) as rowbuf,
            tc.tile_pool(name="psum", bufs=1, space="PSUM") as psum,
        ):
            # ---------------- constants -------------------------------------
            ident_sb = small.tile([128, 128], dt.float32)
            nc.sync.dma_start(out=ident_sb, in_=ident[:, :])
            ones_bf = small.tile([128, 128], dt.bfloat16)
            nc.vector.memset(ones_bf, 1.0)

            # ---------------- prd-side prep ---------------------------------
            # sumsq per prd row i in FREE layout: square prdT chunks (bf16),
            # ones-matmul accumulates over d -> psum[*, i] (bcast over
            # partitions). Then r_i = sqrt(1/ss) and 99*n_i = sqrt(9801*ss).
            pr_ss_ps = psum.tile([128, RPC], dt.float32, tag="p", bufs=7)
            prdT_st = []
            for ch in range(DCH):
                pt_sb = stream.tile([128, RPC], dt.float32, tag=f"ptc{ch}", bufs=1)
                nc.sync.dma_start(out=pt_sb, in_=prdT[ch * 128 : (ch + 1) * 128, :])
                prdT_st.append(pt_sb)
                psq = stream.tile([128, RPC], dt.bfloat16, tag="psq", bufs=1)
                nc.vector.tensor_tensor(out=psq, in0=pt_sb, in1=pt_sb, op=OP.mult)
                nc.tensor.matmul(
                    pr_ss_ps, ones_bf[:, :], psq[:, :],
                    start=(ch == 0), stop=(ch == DCH - 1),
                )
            r_free = small.tile([128, RPC], dt.float32)   # 1/||row i||, free axis
            _act_raw(nc, mybir, out=r_free, in_=pr_ss_ps, func=AF.Rsqrt)
            # 99*I split as 96*I + 3*I (both exact in fp8e4) for the fp8 W inject
            diag96 = small.tile([128, 128], dt.float8e4)
            nc.vector.tensor_scalar(
                out=diag96, in0=ident_sb, scalar1=96.0, scalar2=None, op0=OP.mult
            )
            diag3 = small.tile([128, 128], dt.float8e4)
            nc.vector.tensor_scalar(
                out=diag3, in0=ident_sb, scalar1=3.0, scalar2=None, op0=OP.mult
            )
            # scale prdT by r_i (free-broadcast)
            prdT_r = []
            for ch in range(DCH):
                pt_r = persist.tile([128, RPC], dt.bfloat16, tag=f"prdT{ch}")
                nc.vector.tensor_tensor(
                    out=pt_r, in0=prdT_st[ch], in1=r_free, op=OP.mult
                )
                prdT_r.append(pt_r)

            # ---------------- det-side prep ---------------------------------
            # bf16 det: DMA straight into the matmul operand tiles, square in
            # 2x mode for norms, then scale by c in place (sq reads serialize
            # before the in-place scale via subregion deps).
            det_r = []
            for ch in range(DCH):
                dr = persist.tile([128, ND], dt.bfloat16, tag=f"detr{ch}")
                det_r.append(dr)
            c_rcp = dettmp.tile([128, ND], dt.bfloat16, tag="crcp")
            HW = ND // 2
            for half in range(2):
                hsl = slice(half * HW, (half + 1) * HW)
                dsqs = []
                for ch in range(DCH):
                    rows = slice(ch * 128, (ch + 1) * 128)
                    nc.sync.dma_start(out=det_r[ch][:, hsl], in_=detT_hi[rows, hsl])
                    dsq = dettmp.tile([128, HW], dt.bfloat16, tag=f"dsq{ch}", bufs=1)
                    nc.vector.tensor_tensor(
                        out=dsq, in0=det_r[ch][:, hsl],
                        in1=det_r[ch][:, hsl], op=OP.mult,
                    )
                    dsqs.append(dsq)
                for j in range(HW // 512):
                    jb = half * (HW // 512) + j
                    sl = slice(jb * 512, (jb + 1) * 512)
                    c_sq_ps = psum.tile([128, 512], dt.float32, tag="csq", bufs=1)
                    for ch in range(DCH):
                        nc.tensor.matmul(
                            c_sq_ps,
                            ones_bf[:, :],
                            dsqs[ch][:, j * 512 : (j + 1) * 512],
                            start=(ch == 0),
                            stop=(ch == DCH - 1),
                        )
                    _act_raw(nc, mybir, out=c_rcp[:, sl], in_=c_sq_ps,
                             func=AF.Rsqrt)
                    for ch in range(DCH):
                        nc.vector.tensor_tensor(
                            out=det_r[ch][:, sl],
                            in0=det_r[ch][:, sl],
                            in1=c_rcp[:, sl],
                            op=OP.mult,
                        )

            # ---------------- main loop -------------------------------------
            # Row-tile pairs, left column-half first for both, so the PE
            # never stalls on the right det half while it still streams in.
            for t0 in range(0, RT, 2):
                pair = [t0, t0 + 1]
                ex_t, o1_t, srelu_t, hsum_t = {}, {}, {}, {}
                for t in pair:
                    ex_t[t] = rowbuf.tile([128, ND + 1], dt.float32, tag="ex", bufs=3, name=f"ex{t}")
                    o1_t[t] = stream.tile([128, ND], dt.float32, tag="o1full", bufs=3, name=f"o1{t}")
                    srelu_t[t] = stream.tile([128, JB], dt.float32, tag="srelu", bufs=3, name=f"sr{t}")
                    hsum_t[t] = stream.tile([128, JB], dt.float32, tag="hsum", bufs=3, name=f"hs{t}")
                for phase in range(2):
                    jbs = range(phase * (JB // 2), (phase + 1) * (JB // 2))
                    for t in pair:
                        ex_sb, o1_sb = ex_t[t], o1_t[t]
                        srelu, hsum = srelu_t[t], hsum_t[t]
                        for jb in jbs:
                            w_sb = stream.tile([128, 512], dt.float8e4, tag="w", bufs=10)
                            nc.gpsimd.dma_start(
                                out=w_sb,
                                in_=wm1[t * 128 : (t + 1) * 128,
                                        jb * 512 : (jb + 1) * 512],
                            )
                            p = psum.tile([128, 512], dt.float32, tag="p", bufs=7)
                            for ch in range(DCH):
                                nc.tensor.matmul(
                                    p,
                                    prdT_r[ch][:, t * 128 : (t + 1) * 128],
                                    det_r[ch][:, jb * 512 : (jb + 1) * 512],
                                    start=(ch == 0),
                                    stop=False,
                                )
                            nc.tensor.matmul(
                                p, diag96[:, :], w_sb[:, :],
                                start=False, stop=False, skip_group_check=True,
                            )
                            nc.tensor.matmul(
                                p, diag3[:, :], w_sb[:, :],
                                start=False, stop=True, skip_group_check=True,
                            )
                            nc.vector.tensor_scalar(
                                out=o1_sb[:, jb * 512 : (jb + 1) * 512],
                                in0=p, scalar1=0.0, scalar2=0.0,
                                op0=OP.max, op1=OP.add,
                                accum_out=srelu[:, jb : jb + 1],
                            )
                            _act_raw(nc, mybir,
                                     out=ex_sb[:, jb * 512 : (jb + 1) * 512],
                                     in_=p, func=AF.Reciprocal,
                                     bias=1.0, scale=-1.0,
                                     accum_out=hsum[:, jb : jb + 1])
                        nc.scalar.dma_start(
                            out=out1[t * 128 : (t + 1) * 128,
                                     phase * (ND // 2) : (phase + 1) * (ND // 2)],
                            in_=o1_sb[:, phase * (ND // 2) : (phase + 1) * (ND // 2)],
                        )
                for t in pair:
                    ex_sb = ex_t[t]
                    srelu, hsum = srelu_t[t], hsum_t[t]
                    rowsum = stream.tile([128, 1], dt.float32, tag="rowsum")
                    nc.vector.tensor_reduce(out=rowsum, in_=hsum, axis=X, op=OP.add)
                    anyrel = stream.tile([128, 1], dt.float32, tag="anyrel")
                    nc.vector.tensor_reduce(out=anyrel, in_=srelu, axis=X, op=OP.add)
                    rs_inv = stream.tile([128, 1], dt.float32, tag="rsinv")
                    nc.vector.reciprocal(out=rs_inv, in_=rowsum)
                    g = stream.tile([128, 1], dt.float32, tag="g")
                    nc.vector.tensor_scalar(
                        out=g, in0=anyrel, scalar1=0.0, scalar2=None, op0=OP.is_gt
                    )
                    s_row = stream.tile([128, 1], dt.float32, tag="srow")
                    nc.vector.tensor_tensor(out=s_row, in0=rs_inv, in1=g, op=OP.mult)
                    supple = stream.tile([128, 1], dt.float32, tag="supple")
                    nc.vector.tensor_scalar(
                        out=supple, in0=g, scalar1=-1.0, scalar2=1.0,
                        op0=OP.mult, op1=OP.add,
                    )
                    for jb in range(JB):
                        nc.vector.tensor_scalar(
                            out=ex_sb[:, jb * 512 : (jb + 1) * 512],
                            in0=ex_sb[:, jb * 512 : (jb + 1) * 512],
                            scalar1=s_row[:, :], scalar2=None, op0=OP.mult,
                        )
                    nc.vector.tensor_copy(ex_sb[:, ND : ND + 1], supple)
                    nc.scalar.dma_start(
                        out=ext[t * 128 : (t + 1) * 128, :], in_=ex_sb
                    )
    _split_excess_waits(nc)
    return nc


def _get_nc():
    if "nc" not in _cache:
        _cache["nc"] = _build()
    return _cache["nc"]


def kernel(embeddings, W, len_prd=LP, head=1):
    from concourse.bass_utils import run_bass_kernel_spmd

    embeddings = np.asarray(embeddings, dtype=np.float32)
    W = np.asarray(W)
    assert int(len_prd) == LP and int(head) == 1
    assert embeddings.shape == (N_TOT, D) and W.shape == (N_TOT, N_TOT)

    nc = _get_nc()

    detT = np.ascontiguousarray(embeddings[LP:].T)           # [D, ND] f32
    detT_hi = detT.astype(ml_dtypes.bfloat16)
    # (W - 1) as bf16 without a float conversion pass: 0 -> -1.0, 1 -> 0.0
    Wb = W[:LP, LP:]
    wm1_bits = np.where(Wb == 0, np.uint8(0xB8), np.uint8(0)).astype(np.uint8)
    wm1 = wm1_bits.view(ml_dtypes.float8_e4m3)               # [LP, ND] fp8
    ident = np.eye(128, dtype=np.float32)

    in_maps = []
    for c in range(C):
        rows = slice(c * RPC, (c + 1) * RPC)
        in_maps.append(
            {
                "prdT": np.ascontiguousarray(embeddings[rows].T),
                "detT_hi": detT_hi,
                "wm1": np.ascontiguousarray(wm1[rows]),
                "ident": ident,
            }
        )

    trace = bool(os.environ.get("BASS_KERNEL_TRACE"))
    if trace:
        _install_ntff_hook_shim()
    res = run_bass_kernel_spmd(
        nc, in_maps, core_ids=list(range(C)), trace=trace
    )
    if trace:
        _cache["exec_time_ns"] = res.exec_time_ns
        _cache["last_results"] = res
        print(f"HW exec time: {res.exec_time_ns} ns", file=sys.stderr)

    out1 = np.concatenate([r["out1"] for r in res.results], axis=0)
    extended = np.concatenate([r["ext"] for r in res.results], axis=0)
    return out1, extended
